# revision 19
# baseline (speedup 1.0000x reference)
"""Causal self-attention (B=4, T=2048, C=1024, H=16, D=64) on 8 TRN2 NeuronCores.

Sharding: core c = 2*b + g handles batch b (0..3) and head-group g (0..1),
i.e. 8 heads per core (4 head-pairs). Column-parallel QKV, row-parallel
c_proj; host sums the two partial outputs per batch.

All matmuls run in bfloat16 (1 cycle/row on the PE at 2.4 GHz — measured
~2.3x faster than f32r on HW) with fp32 PSUM accumulation. x and all
weights are cast to bf16 on the host and DMAed directly into their
persistent SBUF layouts (no on-chip staging/casts, half the DMA bytes).
x is fully SBUF-resident (4 MB bf16 = 32 KB/partition).

Per-core pipeline (chunk-pipelined A->B->C over 512-token chunks):
  A(n): QKV projection for chunk n. q/k bias-added on DVE -> bf16 tiles
        (k laid out [head-pair dims, tokens]); v copied -> bf16
        [tokens, head, dim(+ones col)].
  B(n): flash attention, scores transposed: per (pair, k-tile) PE does
        2 QK matmuls (d=64 contraction, heads in partitions 0:64/64:128),
        tri-mask add on diagonal tiles (DVE), exp on ACT (PSUM -> bf16),
        AV with ones-augmented V (M=65) accumulating yT + denominator.
        AV(j) is emitted after QK(j+1) so the in-order PE queue never
        stalls on exp(j). Deferred normalization: reciprocal_approx_fast
        (DVE, input staged to a partition-0 SBUF tile) +
        partition_broadcast (Pool) + multiply (DVE).
  C(n): row-parallel c_proj partial; PSUM -> SBUF copy on DVE, DMA out.

PSUM: 2 banks rotate between A/C projection groups (shared pool, kills
the group-boundary WAR stall), 2x2 banks for score tiles, 2 for psy.
NTFF-profiled: 323 us/exec across 8 cores (f32r baseline: 519 us),
rel err vs fp32 reference 5.6e-3.

Biases: b_attn q/k parts added on-device; v-bias and b_proj folded into
a host-side output correction (softmax rows sum to 1 -> y gets +b_v).
"""

import numpy as np

import concourse.bass as bass
from concourse import bacc, tile, mybir, bass_utils

P = 128
T = 2048
C = 1024
NH = 16          # total heads
D = 64
NCORES = 8
NCH = 4          # 512-token chunks
QC = 512
NKT = T // P     # 16 k tiles
f32 = mybir.dt.float32
bf16 = mybir.dt.bfloat16
Exp = mybir.ActivationFunctionType.Exp
Copy = mybir.ActivationFunctionType.Copy
ADD = mybir.AluOpType.add
MUL = mybir.AluOpType.mult

_CACHE = {}


def _build(REPS=None):
    nc = bacc.Bacc("TRN2", target_bir_lowering=False, debug=False)
    xT = nc.dram_tensor("xT", [C, T], bf16, kind="ExternalInput").ap()
    wqkT = nc.dram_tensor("wqkT", [C, 1024], bf16, kind="ExternalInput").ap()
    wvT = nc.dram_tensor("wvT", [C, 512], bf16, kind="ExternalInput").ap()
    wpT = nc.dram_tensor("wpT", [512, C], bf16, kind="ExternalInput").ap()
    bqk = nc.dram_tensor("bqk", [1024, 1], f32, kind="ExternalInput").ap()
    tri = nc.dram_tensor("tri", [P, P], f32, kind="ExternalInput").ap()
    out = nc.dram_tensor("out", [T, C], f32, kind="ExternalOutput").ap()

    with tile.TileContext(nc) as tc:
        with tc.tile_pool(name="pers", bufs=1) as pers, \
             tc.tile_pool(name="qpool", bufs=2) as qpool, \
             tc.tile_pool(name="epool", bufs=3) as epool, \
             tc.tile_pool(name="fin", bufs=2) as fin, \
             tc.tile_pool(name="ypool", bufs=2) as ypool, \
             tc.tile_pool(name="opool", bufs=2) as opool, \
             tc.tile_pool(name="ac_ps", bufs=2, space="PSUM") as ac_ps, \
             tc.tile_pool(name="qk_ps", bufs=2, space="PSUM") as qk_ps, \
             tc.tile_pool(name="yA_ps", bufs=1, space="PSUM") as yA_ps, \
             tc.tile_pool(name="yB_ps", bufs=1, space="PSUM") as yB_ps:

            # ---- persistent bf16 weights / activations (direct DMA dst) ----
            wqk_sb = [pers.tile([P, 1024], bf16, tag=f"wqk{s}", name=f"wqk{s}")
                      for s in range(8)]
            wv_sb = [pers.tile([P, 512], bf16, tag=f"wv{s}", name=f"wv{s}")
                     for s in range(8)]
            wp_sb = [pers.tile([P, 1024], bf16, tag=f"wp{s}", name=f"wp{s}")
                     for s in range(4)]
            x_sb = pers.tile([P, 8, T], bf16, tag="x", name="x")
            k_sb = [pers.tile([P, T], bf16, tag=f"k{p}", name=f"k{p}") for p in range(4)]
            v_sb = [pers.tile([P, 8, 65], bf16, tag=f"v{t}", name=f"v{t}")
                    for t in range(NKT)]

            def load_weights_qkv():
                # wqk split per (m, s) 128-column slice, ordered to match
                # A(0)'s consumption order (qk(0), qk(4), v, qk(1), qk(5),
                # ...) so the first matmul can start ~1us in.
                for m in (0, 4):
                    for s in range(8):
                        nc.sync.dma_start(wqk_sb[s][:, m * P:(m + 1) * P],
                                          wqkT[s * P:(s + 1) * P, m * P:(m + 1) * P])
                for s in range(8):
                    nc.sync.dma_start(wv_sb[s][:], wvT[s * P:(s + 1) * P, :])
                for lo, hi in ((1, 4), (5, 8)):
                    for s in range(8):
                        nc.sync.dma_start(wqk_sb[s][:, lo * P:hi * P],
                                          wqkT[s * P:(s + 1) * P, lo * P:hi * P])

            def load_x():
                # chunk-0 slices first so A(0) can start ~1us in; the
                # remaining three chunks follow as one DMA per s-block.
                for s in range(8):
                    nc.gpsimd.dma_start(x_sb[:, s, 0:QC], xT[s * P:(s + 1) * P, 0:QC])
                for s in range(8):
                    nc.gpsimd.dma_start(x_sb[:, s, QC:], xT[s * P:(s + 1) * P, QC:])

            def load_weights_proj():
                for s in range(4):
                    nc.sync.dma_start(wp_sb[s][:], wpT[s * P:(s + 1) * P, :])
                    yield

            bqk_sb = pers.tile([P, 8], f32)
            tri_sb = pers.tile([P, P], f32)

            def load_small():
                nc.sync.dma_start(bqk_sb[:], bqk.rearrange("(m p) o -> p (m o)", p=P))
                nc.sync.dma_start(tri_sb[:], tri)

            ones_sb = pers.tile([P, 8], bf16)
            warm_sb = pers.tile([1, 1], f32)

            def init_consts():
                nc.vector.memset(ones_sb[:], 1.0)
                # ones column of every v tile is persistent: set once
                for t in range(NKT):
                    nc.vector.tensor_copy(v_sb[t][:, :, 64:65], ones_sb[:, :, None])
                # preload the exp table set during the prologue
                nc.scalar.activation(warm_sb[:], ones_sb[0:1, 0:1], Exp)

            q_tiles = {}   # (p, n) -> tile
            y_tiles = {}   # (p, n) -> tile

            def phase_a(n):
                def emit_qk(m):
                    ps = ac_ps.tile([P, QC], f32, tag="acps")
                    for s in range(8):
                        nc.tensor.matmul(ps[:], wqk_sb[s][:, m * P:(m + 1) * P],
                                         x_sb[:, s, n * QC:(n + 1) * QC],
                                         start=(s == 0), stop=(s == 7))
                        if s == 3:
                            yield
                    if m < 4:
                        qt = qpool.tile([P, QC], bf16, tag=f"q{m}")
                        nc.vector.tensor_scalar_add(qt[:], ps[:], bqk_sb[:, m:m + 1])
                        q_tiles[(m, n)] = qt
                    else:
                        nc.vector.tensor_scalar_add(k_sb[m - 4][:, n * QC:(n + 1) * QC],
                                                    ps[:], bqk_sb[:, m:m + 1])
                    yield

                def emit_v(ti):
                    t = 4 * n + ti
                    ps = ac_ps.tile([P, 8, D], f32, tag="acps")
                    for s in range(8):
                        nc.tensor.matmul(ps[:], x_sb[:, s, t * P:(t + 1) * P],
                                         wv_sb[s][:], start=(s == 0), stop=(s == 7))
                        if s == 3:
                            yield
                    nc.vector.tensor_copy(v_sb[t][:, :, 0:64], ps[:])
                    yield

                # pair-0 q/k first, then v tiles, then remaining pairs:
                # B(n) pair p unblocks as early as possible.
                yield from emit_qk(0)
                yield from emit_qk(4)
                for ti in range(4):
                    yield from emit_v(ti)
                for p in range(1, 4):
                    yield from emit_qk(p)
                    yield from emit_qk(4 + p)

            def phase_b(n):
                for p in range(4):
                    psy = [
                        yA_ps.tile([65, QC], f32, tag="psyA", name=f"psyA_{n}_{p}"),
                        yB_ps.tile([65, QC], f32, tag="psyB", name=f"psyB_{n}_{p}"),
                    ]
                    last = 4 * n + 3
                    qt = q_tiles[(p, n)]
                    # software-pipelined: AV(j) is emitted after QK(j+1) so
                    # the in-order PE queue never stalls on exp(j) (ACT) --
                    # while ACT computes exp(j), the PE runs QK(j+1).
                    pend = None  # (j, o, e) awaiting AV emission

                    def emit_av(j, o, e):
                        for h in range(2):
                            nc.tensor.matmul(psy[h][:, o:], v_sb[j][:, 2 * p + h, :],
                                             e[:, h, o:], start=(j == 0), stop=(j == last))

                    for j in range(4 * n + 4):
                        diag = j >= 4 * n
                        o = P * (j - 4 * n) if diag else 0
                        ps_g = qk_ps.tile([P, 2, QC], f32, tag="qkg")
                        for h in range(2):
                            b0 = h * 64
                            nc.tensor.matmul(ps_g[:, h, o:], k_sb[p][b0:b0 + 64, j * P:(j + 1) * P],
                                             qt[b0:b0 + 64, o:], start=True, stop=True)
                        if pend is not None:
                            emit_av(*pend)
                        if diag:
                            nc.vector.tensor_tensor(
                                ps_g[:, :, o:o + P], ps_g[:, :, o:o + P],
                                tri_sb[:, None, :].to_broadcast((P, 2, P)), ADD)
                        e = epool.tile([P, 2, QC], bf16, tag="e")
                        nc.scalar.activation(e[:, :, o:], ps_g[:, :, o:], Exp)
                        pend = (j, o, e)
                        yield
                    emit_av(*pend)
                    # finalize: reciprocal of the denominator row (staged to
                    # a partition-0 SBUF tile -- the custom-DVE op only reads
                    # partition 0), partition-0 broadcast (the only broadcast
                    # shape correct on HW), then normalize straight from PSUM.
                    yt = ypool.tile([P, QC], bf16, tag=f"y{p}")
                    dcat = fin.tile([1, 2, QC], f32, tag="d")
                    for h in range(2):
                        nc.vector.tensor_copy(dcat[:, h, :], psy[h][64:65, :])
                    rcat = fin.tile([1, 2, QC], f32, tag="r")
                    nc.vector.reciprocal_approx_fast(rcat[:], dcat[:])
                    rb2 = fin.tile([64, 2, QC], f32, tag="rb")
                    nc.gpsimd.partition_broadcast(rb2[:], rcat[:])
                    for h in range(2):
                        nc.vector.tensor_tensor(yt[h * 64:(h + 1) * 64, :],
                                                psy[h][0:64, :], rb2[:, h, :], MUL)
                    y_tiles[(p, n)] = yt
                    yield

            def phase_c(n):
                for ti in range(4):
                    t = 4 * n + ti
                    for cc in range(2):
                        ps = ac_ps.tile([P, QC], f32, tag="acps")
                        for s in range(4):
                            nc.tensor.matmul(ps[:], y_tiles[(s, n)][:, ti * P:(ti + 1) * P],
                                             wp_sb[s][:, cc * QC:(cc + 1) * QC],
                                             start=(s == 0), stop=(s == 3))
                        ob = opool.tile([P, QC], f32, tag="ob")
                        nc.vector.tensor_copy(ob[:], ps[:])
                        nc.sync.dma_start(out[t * P:(t + 1) * P, cc * QC:(cc + 1) * QC], ob[:])
                        yield

            def chain(*gens):
                for g in gens:
                    yield from g

            def run_all(g):
                for _ in g:
                    pass

            _SENTINEL = object()

            def interleave_lazy(base, inject, rate):
                """Emit all of `base`; after each base step emit `rate` steps
                of `inject` (fractional rates accumulate). Leftover inject
                steps are emitted at the end."""
                inj_iter = iter(inject)
                acc = 0.0
                exhausted = False
                for _ in base:
                    if exhausted:
                        continue
                    acc += rate
                    while acc >= 1.0 and not exhausted:
                        acc -= 1.0
                        if next(inj_iter, _SENTINEL) is _SENTINEL:
                            exhausted = True
                for _ in inj_iter:
                    pass

            # ---- emission schedule ----
            # prologue: all input DMAs queued in consumption order (small
            # consts first, x chunk-0 + early wqk slices next), then A(0)
            # up through pair-0 q/k and the first two v tiles.
            load_small()
            load_x()
            load_weights_qkv()
            init_consts()
            a0 = phase_a(0)
            for _ in range(8):
                next(a0)
            # B(n) yields: 4 * (4n+4 j-steps + 1 finalize)
            b_steps = [4 * (4 * n + 5) for n in range(4)]
            interleave_lazy(phase_b(0), chain(a0, load_weights_proj(), phase_a(1)),
                            (16 + 4 + 24) / b_steps[0])
            interleave_lazy(phase_b(1), chain(phase_a(2), phase_c(0)), 32 / b_steps[1])
            interleave_lazy(phase_b(2), chain(phase_a(3), phase_c(1)), 32 / b_steps[2])
            interleave_lazy(phase_b(3), phase_c(2), 8 / b_steps[3])
            run_all(phase_c(3))

    nc.compile()
    return nc


def _bf16(a):
    import ml_dtypes
    return np.ascontiguousarray(a).astype(ml_dtypes.bfloat16)


def _prep_core_inputs(c, x, w_attn, b_attn):
    b, g = divmod(c, 2)
    heads = [g * 8 + 2 * p + e for p in range(4) for e in range(2)]
    qrows = np.concatenate([np.arange(h * D, (h + 1) * D) for h in heads])
    # wqkT columns: q pairs (scaled 1/8) then k pairs
    wq = w_attn[qrows, :] * 0.125
    wk = w_attn[C + qrows, :]
    wqkT = np.concatenate([wq, wk], 0).T
    wvT = w_attn[2 * C + qrows, :].T
    bqk = np.concatenate([b_attn[qrows] * 0.125, b_attn[C + qrows]]).reshape(1024, 1)
    return {
        "xT": _bf16(x[b].T),
        "wqkT": _bf16(wqkT),
        "wvT": _bf16(wvT),
        "bqk": bqk.astype(np.float32),
    }


def _prep_proj(c, w_proj):
    g = c % 2
    heads = [g * 8 + 2 * p + e for p in range(4) for e in range(2)]
    ch = np.concatenate([np.arange(h * D, (h + 1) * D) for h in heads])
    return _bf16(w_proj[:, ch].T)


def _tri_mask():
    k = np.arange(P)[:, None]
    q = np.arange(P)[None, :]
    return np.where(q >= k, 0.0, -1e30).astype(np.float32)


def kernel(x, w_attn, b_attn, w_proj, b_proj):
    x = np.asarray(x, dtype=np.float32)
    w_attn = np.asarray(w_attn, dtype=np.float32)
    b_attn = np.asarray(b_attn, dtype=np.float32)
    w_proj = np.asarray(w_proj, dtype=np.float32)
    b_proj = np.asarray(b_proj, dtype=np.float32)

    if "nc" not in _CACHE:
        _CACHE["nc"] = _build()
    nc = _CACHE["nc"]

    tri = _tri_mask()
    in_maps = []
    for c in range(NCORES):
        m = _prep_core_inputs(c, x, w_attn, b_attn)
        m["wpT"] = _prep_proj(c, w_proj)
        m["tri"] = tri
        in_maps.append(m)

    res = bass_utils.run_bass_kernel_spmd(nc, in_maps, core_ids=list(range(NCORES)))
    outs = [r["out"] for r in res.results]

    B = x.shape[0]
    corr = (b_attn[2 * C:] @ w_proj.T + b_proj).astype(np.float32)
    full = np.empty((B, T, C), np.float32)
    for b in range(B):
        full[b] = outs[2 * b] + outs[2 * b + 1] + corr
    return full


# revision 22
# speedup vs baseline: 1.0179x; 1.0179x over previous
"""Causal self-attention (B=4, T=2048, C=1024, H=16, D=64) on 8 TRN2 NeuronCores.

Sharding: core c = 2*b + g handles batch b (0..3) and head-group g (0..1),
i.e. 8 heads per core (4 head-pairs). Column-parallel QKV, row-parallel
c_proj; host sums the two partial outputs per batch.

All matmuls run in bfloat16 (1 cycle/row on the PE at 2.4 GHz — measured
~2.3x faster than f32r on HW) with fp32 PSUM accumulation. x and all
weights are cast to bf16 on the host and DMAed directly into their
persistent SBUF layouts (no on-chip staging/casts, half the DMA bytes).
x is fully SBUF-resident (4 MB bf16 = 32 KB/partition).

Per-core pipeline (chunk-pipelined A->B->C over 512-token chunks):
  A(n): QKV projection for chunk n. q/k bias-added on DVE -> bf16 tiles
        (k laid out [head-pair dims, tokens]); v copied -> bf16
        [tokens, head, dim(+ones col)].
  B(n): flash attention, scores transposed: per (pair, k-tile) PE does
        2 QK matmuls (d=64 contraction, heads in partitions 0:64/64:128),
        tri-mask add on diagonal tiles (DVE), exp on ACT (PSUM -> bf16),
        AV with ones-augmented V (M=65) accumulating yT + denominator.
        AV(j) is emitted after QK(j+1) so the in-order PE queue never
        stalls on exp(j). Deferred normalization: reciprocal_approx_fast
        (DVE, input staged to a partition-0 SBUF tile) +
        partition_broadcast (Pool) + multiply (DVE).
  C(n): row-parallel c_proj partial; PSUM -> SBUF copy on DVE, DMA out.

PSUM: 2 banks rotate between A/C projection groups (shared pool, kills
the group-boundary WAR stall), 2x2 banks for score tiles, 2 for psy.
NTFF-profiled: 313 us/exec across 8 cores (f32r baseline: 519 us),
rel err vs fp32 reference 5.6e-3.

Biases: b_attn q/k parts added on-device; v-bias and b_proj folded into
a host-side output correction (softmax rows sum to 1 -> y gets +b_v).
"""

import numpy as np

import concourse.bass as bass
from concourse import bacc, tile, mybir, bass_utils

P = 128
T = 2048
C = 1024
NH = 16          # total heads
D = 64
NCORES = 8
NCH = 4          # 512-token chunks
QC = 512
NKT = T // P     # 16 k tiles
f32 = mybir.dt.float32
bf16 = mybir.dt.bfloat16
Exp = mybir.ActivationFunctionType.Exp
Copy = mybir.ActivationFunctionType.Copy
ADD = mybir.AluOpType.add
MUL = mybir.AluOpType.mult

_CACHE = {}


def _build(REPS=None):
    nc = bacc.Bacc("TRN2", target_bir_lowering=False, debug=False)
    xT = nc.dram_tensor("xT", [C, T], bf16, kind="ExternalInput").ap()
    wqkT = nc.dram_tensor("wqkT", [C, 1024], bf16, kind="ExternalInput").ap()
    wvT = nc.dram_tensor("wvT", [C, 512], bf16, kind="ExternalInput").ap()
    wpT = nc.dram_tensor("wpT", [512, C], bf16, kind="ExternalInput").ap()
    bqk = nc.dram_tensor("bqk", [1024, 1], f32, kind="ExternalInput").ap()
    tri = nc.dram_tensor("tri", [P, P], f32, kind="ExternalInput").ap()
    out = nc.dram_tensor("out", [T, C], f32, kind="ExternalOutput").ap()

    with tile.TileContext(nc) as tc:
        with tc.tile_pool(name="pers", bufs=1) as pers, \
             tc.tile_pool(name="qpool", bufs=2) as qpool, \
             tc.tile_pool(name="epool", bufs=3) as epool, \
             tc.tile_pool(name="fin", bufs=2) as fin, \
             tc.tile_pool(name="ypool", bufs=2) as ypool, \
             tc.tile_pool(name="opool", bufs=2) as opool, \
             tc.tile_pool(name="ac_ps", bufs=2, space="PSUM") as ac_ps, \
             tc.tile_pool(name="qk_ps", bufs=2, space="PSUM") as qk_ps, \
             tc.tile_pool(name="yA_ps", bufs=1, space="PSUM") as yA_ps, \
             tc.tile_pool(name="yB_ps", bufs=1, space="PSUM") as yB_ps:

            # ---- persistent bf16 weights / activations (direct DMA dst) ----
            wqk_sb = [pers.tile([P, 1024], bf16, tag=f"wqk{s}", name=f"wqk{s}")
                      for s in range(8)]
            wv_sb = [pers.tile([P, 512], bf16, tag=f"wv{s}", name=f"wv{s}")
                     for s in range(8)]
            wp_sb = [pers.tile([P, 1024], bf16, tag=f"wp{s}", name=f"wp{s}")
                     for s in range(4)]
            x_sb = pers.tile([P, 8, T], bf16, tag="x", name="x")
            k_sb = [pers.tile([P, T], bf16, tag=f"k{p}", name=f"k{p}") for p in range(4)]
            v_sb = [pers.tile([P, 8, 65], bf16, tag=f"v{t}", name=f"v{t}")
                    for t in range(NKT)]

            def load_weights_qkv():
                # full 2KB-row DMAs (strided narrow slices are ~3x slower);
                # A(0)'s s-chain consumes wqk_sb[s] in arrival order, and
                # m=0..7 all ride in the same row, so qk(4).. wait on nothing.
                for s in range(8):
                    nc.sync.dma_start(wqk_sb[s][:], wqkT[s * P:(s + 1) * P, :])
                for s in range(8):
                    nc.sync.dma_start(wv_sb[s][:], wvT[s * P:(s + 1) * P, :])

            def load_x():
                # chunk-0 slices first so A(0) can start ~1us in; the
                # remaining three chunks follow as one DMA per s-block.
                for s in range(8):
                    nc.gpsimd.dma_start(x_sb[:, s, 0:QC], xT[s * P:(s + 1) * P, 0:QC])
                for s in range(8):
                    nc.gpsimd.dma_start(x_sb[:, s, QC:], xT[s * P:(s + 1) * P, QC:])

            def load_weights_proj():
                for s in range(4):
                    nc.sync.dma_start(wp_sb[s][:], wpT[s * P:(s + 1) * P, :])
                    yield

            bqk_sb = pers.tile([P, 8], f32)
            tri_sb = pers.tile([P, P], f32)

            def load_small():
                nc.sync.dma_start(bqk_sb[:], bqk.rearrange("(m p) o -> p (m o)", p=P))
                nc.sync.dma_start(tri_sb[:], tri)

            ones_sb = pers.tile([P, 8], bf16)
            warm_sb = pers.tile([1, 1], f32)

            def init_consts():
                nc.vector.memset(ones_sb[:], 1.0)
                # ones column of every v tile is persistent: set once
                for t in range(NKT):
                    nc.vector.tensor_copy(v_sb[t][:, :, 64:65], ones_sb[:, :, None])
                # preload the exp table set during the prologue
                nc.scalar.activation(warm_sb[:], ones_sb[0:1, 0:1], Exp)

            q_tiles = {}   # (p, n) -> tile
            y_tiles = {}   # (p, n) -> tile

            def phase_a(n):
                def emit_qk(m):
                    ps = ac_ps.tile([P, QC], f32, tag="acps")
                    for s in range(8):
                        nc.tensor.matmul(ps[:], wqk_sb[s][:, m * P:(m + 1) * P],
                                         x_sb[:, s, n * QC:(n + 1) * QC],
                                         start=(s == 0), stop=(s == 7))
                        if s == 3:
                            yield
                    if m < 4:
                        qt = qpool.tile([P, QC], bf16, tag=f"q{m}")
                        nc.vector.tensor_scalar_add(qt[:], ps[:], bqk_sb[:, m:m + 1])
                        q_tiles[(m, n)] = qt
                    else:
                        nc.vector.tensor_scalar_add(k_sb[m - 4][:, n * QC:(n + 1) * QC],
                                                    ps[:], bqk_sb[:, m:m + 1])
                    yield

                def emit_v(ti):
                    t = 4 * n + ti
                    ps = ac_ps.tile([P, 8, D], f32, tag="acps")
                    for s in range(8):
                        nc.tensor.matmul(ps[:], x_sb[:, s, t * P:(t + 1) * P],
                                         wv_sb[s][:], start=(s == 0), stop=(s == 7))
                        if s == 3:
                            yield
                    nc.vector.tensor_copy(v_sb[t][:, :, 0:64], ps[:])
                    yield

                # pair-0 q/k first, then v tiles, then remaining pairs:
                # B(n) pair p unblocks as early as possible.
                yield from emit_qk(0)
                yield from emit_qk(4)
                for ti in range(4):
                    yield from emit_v(ti)
                for p in range(1, 4):
                    yield from emit_qk(p)
                    yield from emit_qk(4 + p)

            def phase_b(n):
                for p in range(4):
                    psy = [
                        yA_ps.tile([65, QC], f32, tag="psyA", name=f"psyA_{n}_{p}"),
                        yB_ps.tile([65, QC], f32, tag="psyB", name=f"psyB_{n}_{p}"),
                    ]
                    last = 4 * n + 3
                    qt = q_tiles[(p, n)]
                    # software-pipelined: AV(j) is emitted after QK(j+1) so
                    # the in-order PE queue never stalls on exp(j) (ACT) --
                    # while ACT computes exp(j), the PE runs QK(j+1).
                    pend = None  # (j, o, e) awaiting AV emission

                    def emit_av(j, o, e):
                        for h in range(2):
                            nc.tensor.matmul(psy[h][:, o:], v_sb[j][:, 2 * p + h, :],
                                             e[:, h, o:], start=(j == 0), stop=(j == last))

                    for j in range(4 * n + 4):
                        diag = j >= 4 * n
                        o = P * (j - 4 * n) if diag else 0
                        ps_g = qk_ps.tile([P, 2, QC], f32, tag="qkg")
                        for h in range(2):
                            b0 = h * 64
                            nc.tensor.matmul(ps_g[:, h, o:], k_sb[p][b0:b0 + 64, j * P:(j + 1) * P],
                                             qt[b0:b0 + 64, o:], start=True, stop=True)
                        if pend is not None:
                            emit_av(*pend)
                        if diag:
                            nc.vector.tensor_tensor(
                                ps_g[:, :, o:o + P], ps_g[:, :, o:o + P],
                                tri_sb[:, None, :].to_broadcast((P, 2, P)), ADD)
                        e = epool.tile([P, 2, QC], bf16, tag="e")
                        nc.scalar.activation(e[:, :, o:], ps_g[:, :, o:], Exp)
                        pend = (j, o, e)
                        yield
                    emit_av(*pend)
                    # finalize: reciprocal of the denominator row (staged to
                    # a partition-0 SBUF tile -- the custom-DVE op only reads
                    # partition 0), partition-0 broadcast (the only broadcast
                    # shape correct on HW), then normalize straight from PSUM.
                    yt = ypool.tile([P, QC], bf16, tag=f"y{p}")
                    # stage psy into SBUF right away (DVE) so the psy PSUM
                    # banks are released in ~1.5us instead of after the full
                    # recip->broadcast->normalize chain; the next pair's
                    # first AV then starts without a WAR stall.
                    dcat = fin.tile([1, 2, QC], f32, tag="d")
                    yc = fin.tile([64, 2, QC], f32, tag="yc")
                    for h in range(2):
                        nc.vector.tensor_copy(dcat[:, h, :], psy[h][64:65, :])
                        nc.vector.tensor_copy(yc[:, h, :], psy[h][0:64, :])
                    rcat = fin.tile([1, 2, QC], f32, tag="r")
                    nc.vector.reciprocal_approx_fast(rcat[:], dcat[:])
                    rb2 = fin.tile([64, 2, QC], f32, tag="rb")
                    nc.gpsimd.partition_broadcast(rb2[:], rcat[:])
                    for h in range(2):
                        nc.vector.tensor_tensor(yt[h * 64:(h + 1) * 64, :],
                                                yc[:, h, :], rb2[:, h, :], MUL)
                    y_tiles[(p, n)] = yt
                    yield

            def phase_c(n):
                for ti in range(4):
                    t = 4 * n + ti
                    for cc in range(2):
                        ps = ac_ps.tile([P, QC], f32, tag="acps")
                        for s in range(4):
                            nc.tensor.matmul(ps[:], y_tiles[(s, n)][:, ti * P:(ti + 1) * P],
                                             wp_sb[s][:, cc * QC:(cc + 1) * QC],
                                             start=(s == 0), stop=(s == 3))
                        ob = opool.tile([P, QC], f32, tag="ob")
                        nc.vector.tensor_copy(ob[:], ps[:])
                        nc.sync.dma_start(out[t * P:(t + 1) * P, cc * QC:(cc + 1) * QC], ob[:])
                        yield

            def chain(*gens):
                for g in gens:
                    yield from g

            def run_all(g):
                for _ in g:
                    pass

            _SENTINEL = object()

            def interleave_lazy(base, inject, rate):
                """Emit all of `base`; after each base step emit `rate` steps
                of `inject` (fractional rates accumulate). Leftover inject
                steps are emitted at the end."""
                inj_iter = iter(inject)
                acc = 0.0
                exhausted = False
                for _ in base:
                    if exhausted:
                        continue
                    acc += rate
                    while acc >= 1.0 and not exhausted:
                        acc -= 1.0
                        if next(inj_iter, _SENTINEL) is _SENTINEL:
                            exhausted = True
                for _ in inj_iter:
                    pass

            # ---- emission schedule ----
            # prologue: all input DMAs queued in consumption order (small
            # consts first, x chunk-0 + early wqk slices next), then A(0)
            # up through pair-0 q/k and the first two v tiles.
            load_small()
            load_x()
            load_weights_qkv()
            init_consts()
            a0 = phase_a(0)
            for _ in range(8):
                next(a0)
            # B(n) yields: 4 * (4n+4 j-steps + 1 finalize)
            b_steps = [4 * (4 * n + 5) for n in range(4)]
            interleave_lazy(phase_b(0), chain(a0, load_weights_proj(), phase_a(1)),
                            (16 + 4 + 24) / b_steps[0])
            interleave_lazy(phase_b(1), chain(phase_a(2), phase_c(0)), 32 / b_steps[1])
            interleave_lazy(phase_b(2), chain(phase_a(3), phase_c(1)), 32 / b_steps[2])
            interleave_lazy(phase_b(3), phase_c(2), 8 / b_steps[3])
            run_all(phase_c(3))

    nc.compile()
    return nc


def _bf16(a):
    import ml_dtypes
    return np.ascontiguousarray(a).astype(ml_dtypes.bfloat16)


def _prep_core_inputs(c, x, w_attn, b_attn):
    b, g = divmod(c, 2)
    heads = [g * 8 + 2 * p + e for p in range(4) for e in range(2)]
    qrows = np.concatenate([np.arange(h * D, (h + 1) * D) for h in heads])
    # wqkT columns: q pairs (scaled 1/8) then k pairs
    wq = w_attn[qrows, :] * 0.125
    wk = w_attn[C + qrows, :]
    wqkT = np.concatenate([wq, wk], 0).T
    wvT = w_attn[2 * C + qrows, :].T
    bqk = np.concatenate([b_attn[qrows] * 0.125, b_attn[C + qrows]]).reshape(1024, 1)
    return {
        "xT": _bf16(x[b].T),
        "wqkT": _bf16(wqkT),
        "wvT": _bf16(wvT),
        "bqk": bqk.astype(np.float32),
    }


def _prep_proj(c, w_proj):
    g = c % 2
    heads = [g * 8 + 2 * p + e for p in range(4) for e in range(2)]
    ch = np.concatenate([np.arange(h * D, (h + 1) * D) for h in heads])
    return _bf16(w_proj[:, ch].T)


def _tri_mask():
    k = np.arange(P)[:, None]
    q = np.arange(P)[None, :]
    return np.where(q >= k, 0.0, -1e30).astype(np.float32)


def kernel(x, w_attn, b_attn, w_proj, b_proj):
    x = np.asarray(x, dtype=np.float32)
    w_attn = np.asarray(w_attn, dtype=np.float32)
    b_attn = np.asarray(b_attn, dtype=np.float32)
    w_proj = np.asarray(w_proj, dtype=np.float32)
    b_proj = np.asarray(b_proj, dtype=np.float32)

    if "nc" not in _CACHE:
        _CACHE["nc"] = _build()
    nc = _CACHE["nc"]

    tri = _tri_mask()
    in_maps = []
    for c in range(NCORES):
        m = _prep_core_inputs(c, x, w_attn, b_attn)
        m["wpT"] = _prep_proj(c, w_proj)
        m["tri"] = tri
        in_maps.append(m)

    res = bass_utils.run_bass_kernel_spmd(nc, in_maps, core_ids=list(range(NCORES)))
    outs = [r["out"] for r in res.results]

    B = x.shape[0]
    corr = (b_attn[2 * C:] @ w_proj.T + b_proj).astype(np.float32)
    full = np.empty((B, T, C), np.float32)
    for b in range(B):
        full[b] = outs[2 * b] + outs[2 * b + 1] + corr
    return full
